# revision 20
# baseline (speedup 1.0000x reference)
"""Trainium2 Bass kernel for nn_AudioSNN: 2-layer spiking NN (snntorch Leaky).

Reference semantics per timestep t (over T=200 steps):
    cur1 = x_t @ w1.T + b1                      # [B, 128]
    m1   = 0.9*m1 + cur1 - (m1_prev > 1)        # reset-by-subtract
    spk1 = (m1 > 1)
    cur2 = spk1 @ w2.T + b2                     # [B, 5]
    m2   = 0.9*m2 + cur2 - (m2_prev > 1)
    out[t] = spk2 = (m2 > 1)

Strategy (pure data-parallel over batch, 8 cores x 1024 batch rows):
  - Transposed layout: states kept as [feature, batch] so H=128 sits on
    SBUF partitions and batch on the free dim.
  - One fused custom DVE op does a whole membrane update in a single
    instruction:  m_new = m*beta - (m > thr) + cur + bias.
  - Spikes are encoded via ACT Sign: sg = sign(1 - m1) = -sign(m1 - 1),
    so spk1 = (1 - sg)/2.  Layer-2 matmul uses lhsT ~ -0.5*w2.T and a
    per-partition bias to reconstruct w2 @ spk1.
  - All matmuls run in fp16 with hi/lo split pairs (x = xh + xl exactly
    to ~2^-22 rel; w likewise), accumulated exactly in fp32 PSUM:
    mm1 = wh@xh + wh@xl + wl@xh (one K=120-stacked pass, two N=512
    halves); mm2 = w2h@sg + w2l@sg (2 passes, col-tiled 4x).
  - Output path (cheap): layer-2 membranes accumulate in a wide
    [128, OB*256] f32 staging tile; once per OB=20-step block one ACT
    Sign op thresholds the whole block into fp8 bytes (-1 <=> spike)
    and one gpsimd SWDGE DMA ships it to DRAM.  This keeps the SP
    queue free for the x-stream, moves 1 byte per (class, step,
    batch) instead of 4, and keeps compute off GPSIMD (whose ~2us
    per-op launch overhead makes per-step Pool ops prohibitive).
  - Timing methodology (test.py): the whole T-step body can be wrapped
    in an on-device hardware loop (reps=R) so one NEFF execution runs
    the kernel R times back-to-back; the wall-clock slope over R is
    pure device time, immune to the ~100ms axon dispatch/RTT noise.

HW notes from optimization attempts (measured on TRN2, do not repeat):
  - The kernel is DVE-busy-bound: per step the m1 update (fp32 PSUM src
    = 1x mode, (120+1024)/0.96 = 1192ns) + m2 update (392ns) + ~100ns
    issue/sem overhead == the measured ~1690ns/step.  PE (~1280ns),
    ACT (~1380ns), DMA (~690ns) all sit below.
  - Fusing m1+m2 into one [128,1280] DVE op is a paper win (-126ns) but
    a real-HW loss: a 1280-wide (3-PSUM-bank) DVE op measures ~4.2us
    vs the expected 1.46us (microbench: width 1024 chain = 1.51us/op,
    width 1280 = 4.17us/op even with small separate tiles).
  - Slicing per-step state out of one big SBUF arena tile is also
    pathological: same chain with arena-sliced in0/out = 7.3us/op.
    Small dedicated per-step tiles (as here) are the fast path.
  - gpsimd SWDGE output DMAs cost ~1us+ of Pool-sequencer time each,
    scaling with descriptor count -- keep them few and large (OB=20).
  - Remaining structural idea (untried on HW): keep beta^-i-scaled m1
    in PSUM so PE accumulation does the decay and DVE only does a
    {0,c}-encoded spike compare; est. ~-14% but output spikes tolerate
    only ~100 flipped bits vs the fp32 reference, and fp16-quantized
    per-step reset/const deposits (2^-12-grade) are ~100x too coarse;
    every deposit must be an exact fp16 hi/lo pair.
"""

import numpy as np

import concourse.bacc as bacc
import concourse.mybir as mybir
import concourse.tile as tile
import concourse.dve_ops as dve_ops
from concourse.dve_ops import DveOp
from concourse.dve_spec import Spec, Src0, Src1, C0, C1, C2, lower as dve_lower
from concourse.dve_uop import DveOpSpec
from concourse.bass_utils import run_bass_kernel_spmd

F32 = mybir.dt.float32
F16 = mybir.dt.float16
F8 = mybir.dt.float8e4

B, T, F, H, C = 8192, 200, 40, 128, 5
NCORES = 8
BL = B // NCORES          # 1024 batch rows per core
BH = BL // 2              # 512 per mm1 column half
BETA, THR = 0.9, 1.0
NG = 4                    # col-tile groups for layer 2
BG = BL // NG             # 256 batch rows per col group
XB = 4                    # timesteps per x DMA batch
OB = 20                   # timesteps per output block (one batched spike
                          # op + one DMA per block: gpsimd launch overhead
                          # ~2us/op makes per-step Pool ops prohibitive)


# --------------------------------------------------------------------------
# Custom DVE op: fused SNN membrane update
# --------------------------------------------------------------------------

def _snn_ref(in0, in1, s0, s1, imm2):
    out = (
        in0.astype(np.float32) * imm2
        - (in0 > s1).astype(np.float32)
        + in1.astype(np.float32)
        + s0
    )
    return out.astype(np.float32)


def _register_snn_op() -> DveOp:
    """out = in0*imm2 - (in0 > s1) + in1 + s0"""
    name = "SNN_MEMBRANE_STEP"
    for op in dve_ops.OPS:
        if op.name == name:
            return op
    body = Src0 * C2 - (Src0 > C1) + Src1 + C0
    spec = Spec(body=body, reference=_snn_ref)
    shas = {}
    for ver in ("v3", "v4"):
        uops = dve_lower(spec, ver=ver)
        shas[ver] = DveOpSpec(name=name, opcode=0, uops=uops, rd1_en=True).sha(ver)
    op = DveOp(name, spec, subdim=False, uops_sha=shas)
    dve_ops.OPS.append(op)
    dve_ops._SUB_OPCODE_FOR_NAME[op.name] = (
        dve_ops._CUSTOM_DVE_ROW_BASE + len(dve_ops.OPS) - 1
    )
    dve_ops.CUSTOM_DVE_SPECS[op.name] = spec
    return op


SNN_OP = _register_snn_op()


# --------------------------------------------------------------------------
# Bass module
# --------------------------------------------------------------------------

def build_module(t_steps: int = T, probe: str = "", reps: int = 0):
    """reps=0: plain kernel.  reps=R>0: wrap the whole T-step body in a
    hardware loop executing it R times back-to-back on device (used for
    dispatch-free timing: one NEFF execution = R kernel passes; membrane
    state carries over between passes, which is timing-neutral since every
    instruction's cost is data-independent and values stay bounded)."""
    ob = OB if t_steps % OB == 0 else min(OB, t_steps)
    assert t_steps % XB == 0 and t_steps % ob == 0
    # buffer rotations (mod 4) must land back on the initial tiles at the
    # loop seam so rep 2+ reads the tile rep 1 last wrote
    assert reps == 0 or t_steps % 20 == 0
    tb = t_steps // XB
    ob_n = t_steps // ob
    nc = bacc.Bacc("TRN2", target_bir_lowering=False, debug=False)

    # x packed for the K-stacked 3-pass mm1: rows 0-39 = xh, rows 40-79
    # = xl, rows 80-119 = xh again (pairs with [wh; wh; wl] on the weight
    # side).  XB steps side by side in the free dim.
    XW = XB * BL
    xq = nc.dram_tensor("xq", [tb, 120, XW], F16, kind="ExternalInput").ap()
    # w1 fp16 triple-K stack [wh; wh; wl]
    w1trip = nc.dram_tensor("w1trip", [120, H], F16, kind="ExternalInput").ap()
    # w2 fp16 pair (padded to 32 cols)
    w2qh = nc.dram_tensor("w2qh", [H, 32], F16, kind="ExternalInput").ap()
    w2ql = nc.dram_tensor("w2ql", [H, 32], F16, kind="ExternalInput").ap()
    bias1 = nc.dram_tensor("bias1", [H, 1], F32, kind="ExternalInput").ap()
    bias2 = nc.dram_tensor("bias2", [128, 1], F32, kind="ExternalInput").ap()
    # out[blk, 32g+c, i2*BG + j] = spk2 (fp8 0/1) for class c, step
    # t = blk*ob + i2, batch b = g*BG + j
    out = nc.dram_tensor(
        "out", [ob_n, 128, ob * BG], F8, kind="ExternalOutput"
    ).ap()

    with tile.TileContext(nc) as tc:
        with (
            tc.tile_pool(name="const", bufs=1) as cpool,
            tc.tile_pool(name="state", bufs=1) as spool,
            tc.tile_pool(name="xin", bufs=8) as xpool,
            tc.tile_pool(name="sgn", bufs=8) as gpool,
            tc.tile_pool(name="stage", bufs=2) as stpool,
            tc.tile_pool(name="m2st", bufs=2) as m2pool,
            tc.tile_pool(name="ps1", bufs=3, space="PSUM") as p1pool,
            tc.tile_pool(name="ps2", bufs=2, space="PSUM") as p2pool,
        ):
            w1t_s = cpool.tile([120, H], F16)
            w2qh_s = cpool.tile([H, 32], F16)
            w2ql_s = cpool.tile([H, 32], F16)
            b1_s = cpool.tile([H, 1], F32)
            b2_s = cpool.tile([128, 1], F32)
            nc.sync.dma_start(w1t_s[:], w1trip[:])
            nc.sync.dma_start(w2qh_s[:], w2qh[:])
            nc.sync.dma_start(w2ql_s[:], w2ql[:])
            nc.sync.dma_start(b1_s[:], bias1[:])
            nc.sync.dma_start(b2_s[:], bias2[:])

            m1_pool_prev = spool.tile([H, BL], F32, tag="m1a")
            nc.gpsimd.memset(m1_pool_prev[:], 0.0)
            m1_pool_alt = spool.tile([H, BL], F32, tag="m1b")
            m1_pool_alt2 = spool.tile([H, BL], F32, tag="m1c")
            m1_pool_alt3 = spool.tile([H, BL], F32, tag="m1d")
            m1_pool_alt4 = spool.tile([H, BL], F32, tag="m1e")
            m1_bufs = [m1_pool_alt, m1_pool_alt2, m1_pool_alt3, m1_pool_alt4,
                       m1_pool_prev]
            m1_prev = m1_pool_prev

            # layer-2 membranes accumulate into a wide per-block staging
            # tile; one batched fp8 spike op + one DMA ships each block
            m2st_prev = spool.tile([128, BG], F32, tag="m2i")
            nc.gpsimd.memset(m2st_prev[:], 0.0)

            p1_st = p2_st = x_st = sg_st = None
            if probe == "no_mm1":
                p1_st = spool.tile([H, BL], F32, tag="p1s")
                nc.gpsimd.memset(p1_st[:], 0.1)
            if probe == "no_mm2":
                p2_st = spool.tile([128, BG], F32, tag="p2s")
                nc.gpsimd.memset(p2_st[:], 0.1)
            if probe == "no_xdma":
                x_st = spool.tile([120, XW], F16, tag="xs")
                nc.sync.dma_start(x_st[:], xq[0])
            if probe == "no_act":
                sg_st = spool.tile([H, BL], F16, tag="sgs")
                nc.gpsimd.memset(sg_st[:], 1.0)

            state = {"m2_prev": m2st_prev[:], "m2st": None, "p2": None}

            def l2_step(tau):
                """Membrane-2 update for step tau (runs one step late so
                the DVE queue never stalls on the ACT->PE chain); at block
                end one batched gpsimd op thresholds the whole block to
                fp8 spikes and one SWDGE DMA ships it."""
                i2 = tau % ob
                if i2 == 0:
                    state["m2st"] = m2pool.tile(
                        [128, ob * BG], F32, tag="m2st", name="m2st"
                    )
                m2 = state["m2st"][:, i2 * BG : (i2 + 1) * BG]
                if probe == "dve_std":
                    nc.vector.scalar_tensor_tensor(
                        out=m2, in0=state["m2_prev"], scalar=BETA,
                        in1=state["p2"][:],
                        op0=mybir.AluOpType.mult, op1=mybir.AluOpType.add,
                    )
                elif probe != "no_dve":
                    nc.vector._custom_dve(
                        SNN_OP, out=m2, in0=state["m2_prev"],
                        in1=state["p2"][:],
                        s0=b2_s[:, 0:1], s1=THR, imm2=BETA,
                    )
                state["m2_prev"] = m2
                # batched spike: s = sign(1 - m2) in fp8 (-1 <=> spike;
                # host decodes byte == 0xB8) on ACT, in two half-block ops
                # so they interleave with the per-step sign chain; gpsimd's
                # ~2us per-op launch overhead rules out Pool compute here
                hb = (ob // 2) * BG
                if i2 == ob // 2 - 1 and probe not in ("no_spk", "no_dve"):
                    state["stage"] = stpool.tile(
                        [128, ob * BG], F8, tag="st", name="stg"
                    )
                    nc.scalar.activation(
                        state["stage"][:, :hb], state["m2st"][:, :hb],
                        mybir.ActivationFunctionType.Sign,
                        bias=THR, scale=-1.0,
                    )
                if i2 == ob - 1 and probe not in ("no_spk", "no_dve"):
                    nc.scalar.activation(
                        state["stage"][:, hb:], state["m2st"][:, hb:],
                        mybir.ActivationFunctionType.Sign,
                        bias=THR, scale=-1.0,
                    )
                    if probe != "no_outdma":
                        nc.gpsimd.dma_start(out[tau // ob], state["stage"][:])

            from contextlib import nullcontext

            def emit_body():
                nonlocal m1_prev
                for t in range(t_steps):
                    k, s = divmod(t, XB)

                    if s == 0:
                        if probe == "no_xdma":
                            xt = x_st
                        else:
                            xt = xpool.tile([120, XW], F16, tag="x")
                            nc.sync.dma_start(xt[:], xq[k])

                    # mm1: cur1 = w1 @ x via one K=120 stacked pass
                    # ([wh; wh; wl] . [xh; xl; xh]), two N=512 halves
                    p1 = (
                        p1_st if probe == "no_mm1"
                        else p1pool.tile([H, BL], F32, tag="p1")
                    )
                    if probe != "no_mm1":
                        for half in (0, BH):
                            nc.tensor.matmul(
                                p1[:, half : half + BH],
                                w1t_s[:],
                                xt[:, s * BL + half : s * BL + half + BH],
                                start=True, stop=True,
                            )

                    # m1 = beta*m1 - (m1 > 1) + cur1 + b1  (ping-pong
                    # buffers so the next step's write doesn't WAR-wait
                    # on ACT's read)
                    m1 = m1_bufs[-1] if probe == "no_dve" else m1_bufs[t % 5]
                    if probe == "dve_std":
                        nc.vector.scalar_tensor_tensor(
                            out=m1[:], in0=m1_prev[:], scalar=BETA, in1=p1[:],
                            op0=mybir.AluOpType.mult, op1=mybir.AluOpType.add,
                        )
                    elif probe != "no_dve":
                        nc.vector._custom_dve(
                            SNN_OP, out=m1[:], in0=m1_prev[:], in1=p1[:],
                            s0=b1_s[:, 0:1], s1=THR, imm2=BETA,
                        )
                    m1_prev = m1

                    # sg = sign(1 - m1) (= -sign(m1-1); spk1 = (1-sg)/2)
                    if probe == "no_act":
                        sg = sg_st
                    else:
                        sg = gpool.tile([H, BL], F16, tag="sg")
                        nc.scalar.activation(
                            sg[:], m1[:], mybir.ActivationFunctionType.Sign,
                            bias=1.0, scale=-1.0,
                        )

                    # cur2: p2[32g+c, j] = -0.5*(w2@sgn1)[c, 256g+j], 2-pass
                    p2 = (
                        p2_st if probe == "no_mm2"
                        else p2pool.tile([128, BG], F32, tag="p2")
                    )
                    for g in () if probe == "no_mm2" else range(NG):
                        gs = sg[:, BG * g : BG * (g + 1)]
                        nc.tensor.matmul(
                            p2[32 * g : 32 * (g + 1), :], w2qh_s[:], gs,
                            start=True, stop=False, tile_position=(0, 32 * g),
                        )
                        nc.tensor.matmul(
                            p2[32 * g : 32 * (g + 1), :], w2ql_s[:], gs,
                            start=False, stop=True, tile_position=(0, 32 * g),
                        )

                    # m2(t-1) update, one step behind
                    if t > 0:
                        l2_step(t - 1)
                    state["p2"] = p2

                l2_step(t_steps - 1)

            if reps:
                with tc.For_i(0, reps, name="rep"):
                    emit_body()
            else:
                emit_body()

    nc.compile()
    return nc


_MODULE_CACHE: dict = {}


def _get_module(t_steps: int = T):
    if t_steps not in _MODULE_CACHE:
        _MODULE_CACHE[t_steps] = build_module(t_steps)
    return _MODULE_CACHE[t_steps]


# --------------------------------------------------------------------------
# Host-side sharding / gather
# --------------------------------------------------------------------------

def _fp16_pair(a):
    hi = a.astype(np.float16)
    lo = (a - hi.astype(np.float32)).astype(np.float16)
    return hi, lo


def make_in_maps(x, w1, b1, w2, b2, t_steps: int = T):
    x = np.asarray(x, dtype=np.float32)
    w1 = np.asarray(w1, dtype=np.float32)
    b1 = np.asarray(b1, dtype=np.float32)
    w2 = np.asarray(w2, dtype=np.float32)
    b2 = np.asarray(b2, dtype=np.float32)
    tb = t_steps // XB

    w1h, w1l = _fp16_pair(w1.T)                           # [F, H] each
    w1trip = np.zeros((120, H), np.float16)
    w1trip[0:F] = w1h
    w1trip[F : 2 * F] = w1h
    w1trip[2 * F : 3 * F] = w1l

    w2nh, w2nl = _fp16_pair((-0.5 * w2).T)                # [H, C]
    w2qh = np.zeros((H, 32), np.float16)
    w2ql = np.zeros((H, 32), np.float16)
    w2qh[:, :C] = w2nh
    w2ql[:, :C] = w2nl
    # effective -0.5*w2.T the PE uses; bias reconstructs w2 @ spk
    w_eff = w2nh.astype(np.float32) + w2nl.astype(np.float32)
    corr = -w_eff.sum(axis=0) + b2

    bias1 = np.ascontiguousarray(b1[:, None])
    bias2 = np.zeros((128, 1), np.float32)
    for g in range(NG):
        bias2[32 * g : 32 * g + C, 0] = corr

    in_maps = []
    for c in range(NCORES):
        xc = x[c * BL : (c + 1) * BL, :t_steps, :]        # [BL, t, F]
        xt_ = xc.transpose(1, 2, 0)                       # [t, F, BL]
        xh16, xl16 = _fp16_pair(xt_)
        trip = np.concatenate([xh16, xl16, xh16], axis=1)  # [t, 120, BL]
        xqc = (
            trip.reshape(tb, XB, 120, BL)
            .transpose(0, 2, 1, 3)
            .reshape(tb, 120, XB * BL)
        )
        in_maps.append(
            {
                "xq": np.ascontiguousarray(xqc),
                "w1trip": w1trip,
                "w2qh": w2qh,
                "w2ql": w2ql,
                "bias1": bias1,
                "bias2": bias2,
            }
        )
    return in_maps


def postprocess(results, t_steps: int = T):
    """results: list of per-core dicts with 'out' [ob_n, 128, OB*BG] fp8
    spikes (0/1 bytes)."""
    outs = []
    for c in range(NCORES):
        r = np.asarray(results[c]["out"])
        by = r.view(np.uint8) if r.dtype != np.uint8 else r
        spk = (by == 0xB8).astype(np.float32)             # fp8 -1.0 bytes
        ob_n = t_steps // OB
        spk = spk.reshape(ob_n, NG, 32, OB, BG)[:, :, :C]  # [ob, g, c, i2, j]
        # -> [t, b, c] with t = ob*OB+i2, b = g*BG+j
        spk = spk.transpose(0, 3, 1, 4, 2).reshape(t_steps, BL, C)
        outs.append(spk)
    return np.concatenate(outs, axis=1)                   # [t, B, C]


def kernel(x, w1, b1, w2, b2):
    nc = _get_module(T)
    in_maps = make_in_maps(x, w1, b1, w2, b2, T)
    res = run_bass_kernel_spmd(nc, in_maps, core_ids=list(range(NCORES)))
    return postprocess(res.results, T)



# revision 21
# speedup vs baseline: 1.0067x; 1.0067x over previous
"""Trainium2 Bass kernel for nn_AudioSNN: 2-layer spiking NN (snntorch Leaky).

Reference semantics per timestep t (over T=200 steps):
    cur1 = x_t @ w1.T + b1                      # [B, 128]
    m1   = 0.9*m1 + cur1 - (m1_prev > 1)        # reset-by-subtract
    spk1 = (m1 > 1)
    cur2 = spk1 @ w2.T + b2                     # [B, 5]
    m2   = 0.9*m2 + cur2 - (m2_prev > 1)
    out[t] = spk2 = (m2 > 1)

Strategy (pure data-parallel over batch, 8 cores x 1024 batch rows):
  - Transposed layout: states kept as [feature, batch] so H=128 sits on
    SBUF partitions and batch on the free dim.
  - One fused custom DVE op does a whole membrane update in a single
    instruction:  m_new = m*beta - (m > thr) + cur + bias.
  - Spikes are encoded via ACT Sign: sg = sign(1 - m1) = -sign(m1 - 1),
    so spk1 = (1 - sg)/2.  Layer-2 matmul uses lhsT ~ -0.5*w2.T and a
    per-partition bias to reconstruct w2 @ spk1.
  - All matmuls run in fp16 with hi/lo split pairs (x = xh + xl exactly
    to ~2^-22 rel; w likewise), accumulated exactly in fp32 PSUM:
    mm1 = wh@xh + wh@xl + wl@xh (one K=120-stacked pass, two N=512
    halves); mm2 = w2h@sg + w2l@sg (2 passes, col-tiled 4x).
  - Output path (cheap): layer-2 membranes accumulate in a wide
    [128, OB*256] f32 staging tile; once per OB=20-step block one ACT
    Sign op thresholds the whole block into fp8 bytes (-1 <=> spike)
    and one gpsimd SWDGE DMA ships it to DRAM.  This keeps the SP
    queue free for the x-stream, moves 1 byte per (class, step,
    batch) instead of 4, and keeps compute off GPSIMD (whose ~2us
    per-op launch overhead makes per-step Pool ops prohibitive).
  - Timing methodology (test.py): the whole T-step body can be wrapped
    in an on-device hardware loop (reps=R) so one NEFF execution runs
    the kernel R times back-to-back; the wall-clock slope over R is
    pure device time, immune to the ~100ms axon dispatch/RTT noise.

HW notes from optimization attempts (measured on TRN2, do not repeat):
  - The kernel is DVE-busy-bound: per step the m1 update (fp32 PSUM src
    = 1x mode, (120+1024)/0.96 = 1192ns) + m2 update (392ns) + ~100ns
    issue/sem overhead == the measured ~1690ns/step.  PE (~1280ns),
    ACT (~1380ns), DMA (~690ns) all sit below.  HW probe deltas agree:
    removing both DVE ops shortens the period by ~765ns/step while
    removing sign1 (ACT) saves only ~96ns -- the ACT/PE/DMA work hides
    almost entirely under the DVE chain.  fp32-PSUM DVE reads cannot go
    2x on TRN2 (single PSUM read port; 16-bit PSUM is TRN3-only), so
    1584ns/step of DVE busy is this architecture's floor.
  - Fusing m1+m2 into one [128,1280] DVE op is a paper win (-126ns) but
    a real-HW loss: a 1280-wide (3-PSUM-bank) DVE op measures ~4.2us
    vs the expected 1.46us (microbench: width 1024 chain = 1.51us/op,
    width 1280 = 4.17us/op even with small separate tiles).
  - Slicing per-step state out of one big SBUF arena tile is also
    pathological: same chain with arena-sliced in0/out = 7.3us/op.
    Small dedicated per-step tiles (as here) are the fast path.
  - gpsimd SWDGE output DMAs cost ~1us+ of Pool-sequencer time each,
    scaling with descriptor count -- keep them few and large (OB=20).
  - Remaining structural idea (untried on HW): keep beta^-i-scaled m1
    in PSUM so PE accumulation does the decay and DVE only does a
    {0,c}-encoded spike compare; est. ~-14% but output spikes tolerate
    only ~100 flipped bits vs the fp32 reference, and fp16-quantized
    per-step reset/const deposits (2^-12-grade) are ~100x too coarse;
    every deposit must be an exact fp16 hi/lo pair.
"""

import numpy as np

import concourse.bacc as bacc
import concourse.mybir as mybir
import concourse.tile as tile
import concourse.dve_ops as dve_ops
from concourse.dve_ops import DveOp
from concourse.dve_spec import Spec, Src0, Src1, C0, C1, C2, lower as dve_lower
from concourse.dve_uop import DveOpSpec
from concourse.bass_utils import run_bass_kernel_spmd

F32 = mybir.dt.float32
F16 = mybir.dt.float16
F8 = mybir.dt.float8e4

B, T, F, H, C = 8192, 200, 40, 128, 5
NCORES = 8
BL = B // NCORES          # 1024 batch rows per core
BH = BL // 2              # 512 per mm1 column half
BETA, THR = 0.9, 1.0
NG = 4                    # col-tile groups for layer 2
BG = BL // NG             # 256 batch rows per col group
XB = 4                    # timesteps per x DMA batch
OB = 20                   # timesteps per output block (one batched spike
                          # op + one DMA per block: gpsimd launch overhead
                          # ~2us/op makes per-step Pool ops prohibitive)


# --------------------------------------------------------------------------
# Custom DVE op: fused SNN membrane update
# --------------------------------------------------------------------------

def _snn_ref(in0, in1, s0, s1, imm2):
    out = (
        in0.astype(np.float32) * imm2
        - (in0 > s1).astype(np.float32)
        + in1.astype(np.float32)
        + s0
    )
    return out.astype(np.float32)


def _register_snn_op() -> DveOp:
    """out = in0*imm2 - (in0 > s1) + in1 + s0"""
    name = "SNN_MEMBRANE_STEP"
    for op in dve_ops.OPS:
        if op.name == name:
            return op
    body = Src0 * C2 - (Src0 > C1) + Src1 + C0
    spec = Spec(body=body, reference=_snn_ref)
    shas = {}
    for ver in ("v3", "v4"):
        uops = dve_lower(spec, ver=ver)
        shas[ver] = DveOpSpec(name=name, opcode=0, uops=uops, rd1_en=True).sha(ver)
    op = DveOp(name, spec, subdim=False, uops_sha=shas)
    dve_ops.OPS.append(op)
    dve_ops._SUB_OPCODE_FOR_NAME[op.name] = (
        dve_ops._CUSTOM_DVE_ROW_BASE + len(dve_ops.OPS) - 1
    )
    dve_ops.CUSTOM_DVE_SPECS[op.name] = spec
    return op


SNN_OP = _register_snn_op()


# --------------------------------------------------------------------------
# Bass module
# --------------------------------------------------------------------------

def build_module(t_steps: int = T, probe: str = "", reps: int = 0):
    """reps=0: plain kernel.  reps=R>0: wrap the whole T-step body in a
    hardware loop executing it R times back-to-back on device (used for
    dispatch-free timing: one NEFF execution = R kernel passes; membrane
    state carries over between passes, which is timing-neutral since every
    instruction's cost is data-independent and values stay bounded)."""
    ob = OB if t_steps % OB == 0 else min(OB, t_steps)
    assert t_steps % XB == 0 and t_steps % ob == 0
    # buffer rotations (mod 4) must land back on the initial tiles at the
    # loop seam so rep 2+ reads the tile rep 1 last wrote
    assert reps == 0 or t_steps % 20 == 0
    tb = t_steps // XB
    ob_n = t_steps // ob
    nc = bacc.Bacc("TRN2", target_bir_lowering=False, debug=False)

    # x packed for the K-stacked 3-pass mm1: rows 0-39 = xh, rows 40-79
    # = xl, rows 80-119 = xh again (pairs with [wh; wh; wl] on the weight
    # side).  XB steps side by side in the free dim.
    XW = XB * BL
    xq = nc.dram_tensor("xq", [tb, 120, XW], F16, kind="ExternalInput").ap()
    # w1 fp16 triple-K stack [wh; wh; wl]
    w1trip = nc.dram_tensor("w1trip", [120, H], F16, kind="ExternalInput").ap()
    # w2 fp16 pair (padded to 32 cols)
    w2qh = nc.dram_tensor("w2qh", [H, 32], F16, kind="ExternalInput").ap()
    w2ql = nc.dram_tensor("w2ql", [H, 32], F16, kind="ExternalInput").ap()
    bias1 = nc.dram_tensor("bias1", [H, 1], F32, kind="ExternalInput").ap()
    bias2 = nc.dram_tensor("bias2", [128, 1], F32, kind="ExternalInput").ap()
    # out[blk, 32g+c, i2*BG + j] = spk2 (fp8 0/1) for class c, step
    # t = blk*ob + i2, batch b = g*BG + j
    out = nc.dram_tensor(
        "out", [ob_n, 128, ob * BG], F8, kind="ExternalOutput"
    ).ap()

    with tile.TileContext(nc) as tc:
        with (
            tc.tile_pool(name="const", bufs=1) as cpool,
            tc.tile_pool(name="state", bufs=1) as spool,
            tc.tile_pool(name="xin", bufs=8) as xpool,
            tc.tile_pool(name="sgn", bufs=8) as gpool,
            tc.tile_pool(name="stage", bufs=2) as stpool,
            tc.tile_pool(name="m2st", bufs=2) as m2pool,
            tc.tile_pool(name="ps1", bufs=3, space="PSUM") as p1pool,
            tc.tile_pool(name="ps2", bufs=2, space="PSUM") as p2pool,
        ):
            w1t_s = cpool.tile([120, H], F16)
            w2qh_s = cpool.tile([H, 32], F16)
            w2ql_s = cpool.tile([H, 32], F16)
            b1_s = cpool.tile([H, 1], F32)
            b2_s = cpool.tile([128, 1], F32)
            nc.sync.dma_start(w1t_s[:], w1trip[:])
            nc.sync.dma_start(w2qh_s[:], w2qh[:])
            nc.sync.dma_start(w2ql_s[:], w2ql[:])
            nc.sync.dma_start(b1_s[:], bias1[:])
            nc.sync.dma_start(b2_s[:], bias2[:])

            m1_pool_prev = spool.tile([H, BL], F32, tag="m1a")
            nc.gpsimd.memset(m1_pool_prev[:], 0.0)
            m1_pool_alt = spool.tile([H, BL], F32, tag="m1b")
            m1_pool_alt2 = spool.tile([H, BL], F32, tag="m1c")
            m1_pool_alt3 = spool.tile([H, BL], F32, tag="m1d")
            m1_pool_alt4 = spool.tile([H, BL], F32, tag="m1e")
            m1_bufs = [m1_pool_alt, m1_pool_alt2, m1_pool_alt3, m1_pool_alt4,
                       m1_pool_prev]
            m1_prev = m1_pool_prev

            # layer-2 membranes accumulate into a wide per-block staging
            # tile; one batched fp8 spike op + one DMA ships each block
            m2st_prev = spool.tile([128, BG], F32, tag="m2i")
            nc.gpsimd.memset(m2st_prev[:], 0.0)

            p1_st = p2_st = x_st = sg_st = None
            if probe == "no_mm1":
                p1_st = spool.tile([H, BL], F32, tag="p1s")
                nc.gpsimd.memset(p1_st[:], 0.1)
            if probe == "no_mm2":
                p2_st = spool.tile([128, BG], F32, tag="p2s")
                nc.gpsimd.memset(p2_st[:], 0.1)
            if probe == "no_xdma":
                x_st = spool.tile([120, XW], F16, tag="xs")
                nc.sync.dma_start(x_st[:], xq[0])
            if probe == "no_act":
                sg_st = spool.tile([H, BL], F16, tag="sgs")
                nc.gpsimd.memset(sg_st[:], 1.0)

            state = {"m2_prev": m2st_prev[:], "m2st": None, "p2": None}

            def l2_step(tau):
                """Membrane-2 update for step tau (runs one step late so
                the DVE queue never stalls on the ACT->PE chain); at block
                end one batched gpsimd op thresholds the whole block to
                fp8 spikes and one SWDGE DMA ships it."""
                i2 = tau % ob
                if i2 == 0:
                    state["m2st"] = m2pool.tile(
                        [128, ob * BG], F32, tag="m2st", name="m2st"
                    )
                m2 = state["m2st"][:, i2 * BG : (i2 + 1) * BG]
                if probe == "dve_std":
                    nc.vector.scalar_tensor_tensor(
                        out=m2, in0=state["m2_prev"], scalar=BETA,
                        in1=state["p2"][:],
                        op0=mybir.AluOpType.mult, op1=mybir.AluOpType.add,
                    )
                elif probe != "no_dve":
                    nc.vector._custom_dve(
                        SNN_OP, out=m2, in0=state["m2_prev"],
                        in1=state["p2"][:],
                        s0=b2_s[:, 0:1], s1=THR, imm2=BETA,
                    )
                state["m2_prev"] = m2
                # batched spike: s = sign(1 - m2) in fp8 (-1 <=> spike;
                # host decodes byte == 0xB8) on ACT, in two half-block ops
                # so they interleave with the per-step sign chain; gpsimd's
                # ~2us per-op launch overhead rules out Pool compute here
                hb = (ob // 2) * BG
                if i2 == ob // 2 - 1 and probe not in ("no_spk", "no_dve"):
                    state["stage"] = stpool.tile(
                        [128, ob * BG], F8, tag="st", name="stg"
                    )
                    nc.scalar.activation(
                        state["stage"][:, :hb], state["m2st"][:, :hb],
                        mybir.ActivationFunctionType.Sign,
                        bias=THR, scale=-1.0,
                    )
                if i2 == ob - 1 and probe not in ("no_spk", "no_dve"):
                    nc.scalar.activation(
                        state["stage"][:, hb:], state["m2st"][:, hb:],
                        mybir.ActivationFunctionType.Sign,
                        bias=THR, scale=-1.0,
                    )
                    if probe != "no_outdma":
                        nc.gpsimd.dma_start(out[tau // ob], state["stage"][:])

            from contextlib import nullcontext

            def emit_body():
                nonlocal m1_prev
                for t in range(t_steps):
                    k, s = divmod(t, XB)

                    if s == 0:
                        if probe == "no_xdma":
                            xt = x_st
                        else:
                            xt = xpool.tile([120, XW], F16, tag="x")
                            nc.sync.dma_start(xt[:], xq[k])

                    # mm1: cur1 = w1 @ x via one K=120 stacked pass
                    # ([wh; wh; wl] . [xh; xl; xh]), two N=512 halves
                    p1 = (
                        p1_st if probe == "no_mm1"
                        else p1pool.tile([H, BL], F32, tag="p1")
                    )
                    if probe != "no_mm1":
                        for half in (0, BH):
                            nc.tensor.matmul(
                                p1[:, half : half + BH],
                                w1t_s[:],
                                xt[:, s * BL + half : s * BL + half + BH],
                                start=True, stop=True,
                            )

                    # m1 = beta*m1 - (m1 > 1) + cur1 + b1  (ping-pong
                    # buffers so the next step's write doesn't WAR-wait
                    # on ACT's read)
                    m1 = m1_bufs[-1] if probe == "no_dve" else m1_bufs[t % 5]
                    if probe == "dve_std":
                        nc.vector.scalar_tensor_tensor(
                            out=m1[:], in0=m1_prev[:], scalar=BETA, in1=p1[:],
                            op0=mybir.AluOpType.mult, op1=mybir.AluOpType.add,
                        )
                    elif probe != "no_dve":
                        nc.vector._custom_dve(
                            SNN_OP, out=m1[:], in0=m1_prev[:], in1=p1[:],
                            s0=b1_s[:, 0:1], s1=THR, imm2=BETA,
                        )
                    m1_prev = m1

                    # sg = sign(1 - m1) (= -sign(m1-1); spk1 = (1-sg)/2)
                    if probe == "no_act":
                        sg = sg_st
                    else:
                        sg = gpool.tile([H, BL], F16, tag="sg")
                        nc.scalar.activation(
                            sg[:], m1[:], mybir.ActivationFunctionType.Sign,
                            bias=1.0, scale=-1.0,
                        )

                    # cur2: p2[32g+c, j] = -0.5*(w2@sgn1)[c, 256g+j], 2-pass
                    p2 = (
                        p2_st if probe == "no_mm2"
                        else p2pool.tile([128, BG], F32, tag="p2")
                    )
                    for g in () if probe == "no_mm2" else range(NG):
                        gs = sg[:, BG * g : BG * (g + 1)]
                        nc.tensor.matmul(
                            p2[32 * g : 32 * (g + 1), :], w2qh_s[:], gs,
                            start=True, stop=False, tile_position=(0, 32 * g),
                        )
                        nc.tensor.matmul(
                            p2[32 * g : 32 * (g + 1), :], w2ql_s[:], gs,
                            start=False, stop=True, tile_position=(0, 32 * g),
                        )

                    # m2(t-1) update, one step behind
                    if t > 0:
                        l2_step(t - 1)
                    state["p2"] = p2

                l2_step(t_steps - 1)

            if reps:
                with tc.For_i(0, reps, name="rep"):
                    emit_body()
            else:
                emit_body()

    nc.compile()
    return nc


_MODULE_CACHE: dict = {}


def _get_module(t_steps: int = T):
    if t_steps not in _MODULE_CACHE:
        _MODULE_CACHE[t_steps] = build_module(t_steps)
    return _MODULE_CACHE[t_steps]


# --------------------------------------------------------------------------
# Host-side sharding / gather
# --------------------------------------------------------------------------

def _fp16_pair(a):
    hi = a.astype(np.float16)
    lo = (a - hi.astype(np.float32)).astype(np.float16)
    return hi, lo


def make_in_maps(x, w1, b1, w2, b2, t_steps: int = T):
    x = np.asarray(x, dtype=np.float32)
    w1 = np.asarray(w1, dtype=np.float32)
    b1 = np.asarray(b1, dtype=np.float32)
    w2 = np.asarray(w2, dtype=np.float32)
    b2 = np.asarray(b2, dtype=np.float32)
    tb = t_steps // XB

    w1h, w1l = _fp16_pair(w1.T)                           # [F, H] each
    w1trip = np.zeros((120, H), np.float16)
    w1trip[0:F] = w1h
    w1trip[F : 2 * F] = w1h
    w1trip[2 * F : 3 * F] = w1l

    w2nh, w2nl = _fp16_pair((-0.5 * w2).T)                # [H, C]
    w2qh = np.zeros((H, 32), np.float16)
    w2ql = np.zeros((H, 32), np.float16)
    w2qh[:, :C] = w2nh
    w2ql[:, :C] = w2nl
    # effective -0.5*w2.T the PE uses; bias reconstructs w2 @ spk
    w_eff = w2nh.astype(np.float32) + w2nl.astype(np.float32)
    corr = -w_eff.sum(axis=0) + b2

    bias1 = np.ascontiguousarray(b1[:, None])
    bias2 = np.zeros((128, 1), np.float32)
    for g in range(NG):
        bias2[32 * g : 32 * g + C, 0] = corr

    in_maps = []
    for c in range(NCORES):
        xc = x[c * BL : (c + 1) * BL, :t_steps, :]        # [BL, t, F]
        xt_ = xc.transpose(1, 2, 0)                       # [t, F, BL]
        xh16, xl16 = _fp16_pair(xt_)
        trip = np.concatenate([xh16, xl16, xh16], axis=1)  # [t, 120, BL]
        xqc = (
            trip.reshape(tb, XB, 120, BL)
            .transpose(0, 2, 1, 3)
            .reshape(tb, 120, XB * BL)
        )
        in_maps.append(
            {
                "xq": np.ascontiguousarray(xqc),
                "w1trip": w1trip,
                "w2qh": w2qh,
                "w2ql": w2ql,
                "bias1": bias1,
                "bias2": bias2,
            }
        )
    return in_maps


def postprocess(results, t_steps: int = T):
    """results: list of per-core dicts with 'out' [ob_n, 128, OB*BG] fp8
    spikes (0/1 bytes)."""
    outs = []
    for c in range(NCORES):
        r = np.asarray(results[c]["out"])
        by = r.view(np.uint8) if r.dtype != np.uint8 else r
        spk = (by == 0xB8).astype(np.float32)             # fp8 -1.0 bytes
        ob_n = t_steps // OB
        spk = spk.reshape(ob_n, NG, 32, OB, BG)[:, :, :C]  # [ob, g, c, i2, j]
        # -> [t, b, c] with t = ob*OB+i2, b = g*BG+j
        spk = spk.transpose(0, 3, 1, 4, 2).reshape(t_steps, BL, C)
        outs.append(spk)
    return np.concatenate(outs, axis=1)                   # [t, B, C]


def kernel(x, w1, b1, w2, b2):
    nc = _get_module(T)
    in_maps = make_in_maps(x, w1, b1, w2, b2, T)
    res = run_bass_kernel_spmd(nc, in_maps, core_ids=list(range(NCORES)))
    return postprocess(res.results, T)

